# revision 2
# baseline (speedup 1.0000x reference)
"""HMM forward-algorithm loss on 8 NeuronCores (Bass/Tile) — segment-parallel.

Math (linear domain): loss_n = log( p0^T  M_1 M_2 ... M_T  1 ),
M_t = Texp diag(e_t), e_t = emission probs of token x[n,t].

T is split into S=32 segments of L=128 steps. Products of >=128 random
stochastic matrices are numerically rank-1, so segment products are joined
through rank-1 telescoping:
    p ~= (F0·B1) * prod_j (Fj·B_{j+1}) / (Fj·1)
where Fj = u^T P_j (forward vector chain, u=p0 or ones) and B_j = P_j 1
(backward chain). All 2(S-1) chains are independent -> fully parallel.

Per core c: segments {c, c+8, c+16, c+24}; one 128-wide forward super-chain
(4 segs x 32 seqs) and one backward super-chain. Emissions come from a
device-built bf16 table tau[v,k] = exp(voc_v·emb_k - C0) stored in a
block-permuted DRAM layout; kappa/s column normalization is folded into the
transition matrices. Renorm every R=16 steps exports raw column sums
(logs taken on host). Host assembles the telescoped loss in float64.
"""

import numpy as np

K = 128
V = 50000
T = 4096
N = 32
S = 32                      # segments
L = T // S                  # 128 steps per segment
R = 8                       # renorm period
NREN = L // R - 1           # 15 renorm boundaries (none at chain end)
KAPPA = float(2 ** 25)
C0 = 40.0
VP = 50176                  # 49 * 1024, zero-padded vocab
NGRP = VP // 1024           # 49 voc DMA groups
TROWS = 16384               # compacted (first-use) table rows
TGRP = TROWS // 1024        # 16 table write groups
NGATH = 8                   # batched gathers (2048 rows each)
GW = L * K // NGATH         # 2048 gather cols per batch

_CACHE = {}
DEBUG_TAPS = False


def _build_nc():
    import concourse.bass as bass
    import concourse.mybir as mybir
    import concourse.tile as tile
    from concourse import bacc
    from concourse.masks import make_identity

    f32 = mybir.dt.float32
    bf16 = mybir.dt.bfloat16
    i32 = mybir.dt.int32
    EXP = mybir.ActivationFunctionType.Exp

    nc = bacc.Bacc("TRN2", target_bir_lowering=False, debug=False, num_devices=8)

    vocT_d = nc.dram_tensor("vocT", [K, VP], bf16, kind="ExternalInput")
    texp_d = nc.dram_tensor("texp", [K, K], f32, kind="ExternalInput")
    texpT_d = nc.dram_tensor("texpT", [K, K], f32, kind="ExternalInput")
    embT_d = nc.dram_tensor("embT", [K, K], f32, kind="ExternalInput")
    qf0_d = nc.dram_tensor("qf0", [K, K], f32, kind="ExternalInput")
    offs_d = nc.dram_tensor("offs", [K, L], i32, kind="ExternalInput")

    qf_out_d = nc.dram_tensor("qf_out", [K, K], f32, kind="ExternalOutput")
    qb_out_d = nc.dram_tensor("qb_out", [K, K], f32, kind="ExternalOutput")
    csfb_out_d = nc.dram_tensor("csfb_out", [1, NREN * 2 * K], f32,
                                kind="ExternalOutput")

    table_d = nc.dram_tensor("table", [TROWS, K], bf16)  # first-use compacted
    if DEBUG_TAPS:
        dbg_gath_d = nc.dram_tensor("dbg_gath", [K, 2048], bf16, kind="ExternalOutput")
        dbg_tab_d = nc.dram_tensor("dbg_tab", [512, K], bf16, kind="ExternalOutput")
        dbg_w_d = nc.dram_tensor("dbg_w", [K, 2 * K], bf16, kind="ExternalOutput")

    with tile.TileContext(nc) as tc:
        with (
            tc.tile_pool(name="csb", bufs=1) as csb,
            tc.tile_pool(name="bvt", bufs=2) as bvt,
            tc.tile_pool(name="bex", bufs=3) as bex,
            tc.tile_pool(name="msb", bufs=1) as msb,
            tc.tile_pool(name="set", bufs=3) as set_,
            tc.tile_pool(name="sqt", bufs=2) as sqt,
            tc.tile_pool(name="sq", bufs=2) as sq,
            tc.tile_pool(name="srn", bufs=2) as srn,
            tc.tile_pool(name="bpl", bufs=2, space="PSUM") as bpl,
            tc.tile_pool(name="bps", bufs=1, space="PSUM") as bps,
            tc.tile_pool(name="spt", bufs=2, space="PSUM") as spt,
            tc.tile_pool(name="spfb", bufs=2, space="PSUM") as spfb,
            tc.tile_pool(name="sbc", bufs=1, space="PSUM") as sbc,
        ):
            # ---------------- constants / small inputs ----------------
            ident = csb.tile([K, K], dtype=f32)
            make_identity(nc, ident[:])
            id_b = csb.tile([K, K], dtype=bf16)
            nc.vector.tensor_copy(out=id_b[:], in_=ident[:])
            ones_col_b = csb.tile([K, 1], dtype=bf16)
            nc.vector.memset(ones_col_b[:], 1.0)
            ones_row_b = csb.tile([1, K], dtype=bf16)
            nc.vector.memset(ones_row_b[:], 1.0)
            ones_b = csb.tile([K, K], dtype=bf16)
            nc.vector.memset(ones_b[:], 1.0)
            negc0 = csb.tile([K, 1], dtype=f32)
            nc.vector.memset(negc0[:], -C0)

            texp = csb.tile([K, K], dtype=f32)
            nc.sync.dma_start(out=texp[:], in_=texp_d[:, :])
            texpT = csb.tile([K, K], dtype=f32)
            nc.sync.dma_start(out=texpT[:], in_=texpT_d[:, :])
            embT = csb.tile([K, K], dtype=f32)
            nc.sync.dma_start(out=embT[:], in_=embT_d[:, :])
            embT_b = csb.tile([K, K], dtype=bf16)
            nc.vector.tensor_copy(out=embT_b[:], in_=embT[:])
            qf0 = csb.tile([K, K], dtype=f32)
            nc.sync.dma_start(out=qf0[:], in_=qf0_d[:, :])
            qf0_b = csb.tile([K, K], dtype=bf16)
            nc.vector.tensor_copy(out=qf0_b[:], in_=qf0[:])
            idx_sb = csb.tile([K, L], dtype=i32)
            nc.sync.dma_start(out=idx_sb[:], in_=offs_d[:, :])

            gath = csb.tile([K, NGATH * GW], dtype=bf16)   # all gathered rows
            csfb_sb = csb.tile([1, NREN * 2 * K], dtype=f32)
            s4 = csb.tile([1, 4 * K], dtype=f32)
            Wf_b = csb.tile([K, K], dtype=bf16)
            Wb_b = csb.tile([K, K], dtype=bf16)

            # ---------------- build machinery ----------------
            s_acc = bps.tile([1, 4 * K], dtype=f32, tag="s")
            vts2_box = [None]

            def emit_group(g):
                if g % 2 == 0:
                    w = 2048 if g + 1 < NGRP else 1024
                    vts2 = bvt.tile([K, 2048], dtype=bf16, tag="vt")
                    vts2_box[0] = vts2
                    nc.sync.dma_start(
                        out=vts2[:, 0:w],
                        in_=vocT_d[:, g * 1024 : g * 1024 + w],
                    )
                vts = vts2_box[0][:, (g % 2) * 1024 : (g % 2 + 1) * 1024]
                ex = bex.tile([K, 1024], dtype=bf16, tag="ex")
                for h in range(2):
                    pl = bpl.tile([K, 4 * K], dtype=f32, tag="pl")
                    for b4 in range(4):
                        nc.tensor.matmul(
                            out=pl[:, b4 * K : (b4 + 1) * K],
                            lhsT=vts[:, (h * 4 + b4) * K : (h * 4 + b4 + 1) * K],
                            rhs=embT_b[:],
                            start=True, stop=True, skip_group_check=True,
                        )
                    nc.scalar.activation(
                        out=ex[:, h * 512 : (h + 1) * 512],
                        in_=pl[:], func=EXP, bias=negc0[:, :1],
                    )
                    nc.tensor.matmul(
                        out=s_acc[:1, :],
                        lhsT=ones_col_b[:, :1],
                        rhs=ex[:, h * 512 : (h + 1) * 512],
                        start=(g == 0 and h == 0),
                        stop=(g == NGRP - 1 and h == 1),
                        skip_group_check=True,
                    )
                if g < TGRP:
                    nc.scalar.dma_start(
                        out=table_d[g * 1024 : (g + 1) * 1024, :], in_=ex[:],
                    )

            # table groups first (unblock gathers asap)
            for g in range(TGRP):
                emit_group(g)

            # ---------------- gathers (Pool queue, self-pacing) ----------
            order = []
            for i in range(L // 2):
                order += [i, L - 1 - i]
            for r in order:
                nc.gpsimd.indirect_dma_start(
                    out=gath[:, r * K : (r + 1) * K],
                    out_offset=None,
                    in_=table_d[:, :],
                    in_offset=bass.IndirectOffsetOnAxis(
                        ap=idx_sb[:, r : r + 1], axis=0
                    ),
                )

            # ---------------- Wf/Wb dependencies come from s which needs all
            # groups; emit W computation AFTER interleave below.

            def e_tile(step):
                return gath[:, step * K : (step + 1) * K]

            # ---------------- W build needs s: emitted once remaining build
            # groups are done; but the first 15 scan rounds need Wf/Wb!
            # So: emit the REMAINING build groups now (they only use PE/ACT
            # which would otherwise idle during gather lead-in), then W, then
            # the scan.
            for g in range(TGRP, NGRP):
                emit_group(g)
            nc.vector.tensor_copy(out=s4[:], in_=s_acc[:1, :])

            s2 = msb.tile([1, 2 * K], dtype=f32, tag="s2")
            nc.vector.tensor_add(
                out=s2[:], in0=s4[:1, 0 : 2 * K], in1=s4[:1, 2 * K : 4 * K]
            )
            s1 = msb.tile([1, K], dtype=f32, tag="s1")
            nc.vector.tensor_add(out=s1[:], in0=s2[:1, 0:K], in1=s2[:1, K : 2 * K])
            rs = msb.tile([1, K], dtype=f32, tag="rs")
            nc.vector.reciprocal(out=rs[:], in_=s1[:])
            rsk_b = msb.tile([1, K], dtype=bf16, tag="rsk")
            nc.vector.tensor_scalar_mul(out=rsk_b[:], in0=rs[:], scalar1=KAPPA)
            ones11_b = msb.tile([1, 1], dtype=bf16, tag="o11")
            nc.vector.memset(ones11_b[:], 1.0)
            bcw = sbc.tile([K, 4 * K], dtype=f32, tag="bc")
            nc.tensor.matmul(
                out=bcw[:, 0:1], lhsT=rsk_b[:1, :], rhs=ones11_b[:1, :],
                start=True, stop=True, skip_group_check=True,
            )
            nc.tensor.matmul(
                out=bcw[:, K : 2 * K], lhsT=ones_row_b[:1, :], rhs=rsk_b[:1, :],
                start=True, stop=True, skip_group_check=True,
            )
            nc.vector.tensor_mul(out=Wf_b[:], in0=texp[:], in1=bcw[:, K : 2 * K])
            kc_sb = msb.tile([K, 1], dtype=f32, tag="kcs")
            nc.vector.tensor_copy(out=kc_sb[:], in_=bcw[:, 0:1])
            nc.vector.tensor_scalar_mul(
                out=Wb_b[:], in0=texpT[:], scalar1=kc_sb[:, :1]
            )

            # ---------------- scan (fused F+B rounds) ----------------
            qf_cur = qf0_b[:]
            tmp_prev = None
            pend = None
            for r in range(L):
                tpp = spt.tile([K, 2 * K], dtype=bf16, tag="tp")
                nc.tensor.matmul(
                    out=tpp[:, 0:K], lhsT=e_tile(r), rhs=id_b[:],
                    is_transpose=True, start=True, stop=True,
                    skip_group_check=True,
                )
                nc.tensor.matmul(
                    out=tpp[:, K : 2 * K], lhsT=e_tile(L - 1 - r), rhs=id_b[:],
                    is_transpose=True, start=True, stop=True,
                    skip_group_check=True,
                )
                ee = set_.tile([K, 2 * K], dtype=bf16, tag="ee")
                nc.scalar.copy(out=ee[:], in_=tpp[:])
                if pend is not None:
                    nc.vector.tensor_mul(out=ee[:], in0=ee[:], in1=pend)
                    pend = None
                pk = spfb.tile([K, 2 * K], dtype=f32, tag="pk")
                nc.tensor.matmul(
                    out=pk[:, 0:K], lhsT=Wf_b[:], rhs=qf_cur,
                    start=True, stop=True, skip_group_check=True,
                )
                if r > 0:
                    nc.tensor.matmul(
                        out=pk[:, K : 2 * K], lhsT=Wb_b[:], rhs=tmp_prev,
                        start=True, stop=True, skip_group_check=True,
                    )
                out2 = sqt.tile([K, 2 * K], dtype=bf16, tag="q2")
                if r == 0:
                    nc.vector.tensor_mul(
                        out=out2[:, 0:K], in0=pk[:, 0:K], in1=ee[:, 0:K]
                    )
                    nc.vector.tensor_mul(
                        out=out2[:, K : 2 * K], in0=ones_b[:],
                        in1=ee[:, K : 2 * K],
                    )
                else:
                    nc.vector.tensor_mul(out=out2[:], in0=pk[:], in1=ee[:])
                qf_cur = out2[:, 0:K]
                tmp_prev = out2[:, K : 2 * K]
                if (r + 1) % R == 0 and r + 1 < L:
                    ridx = (r + 1) // R - 1
                    bcpair = sbc.tile([K, 4 * K], dtype=f32, tag="bc")
                    nc.tensor.matmul(
                        out=bcpair[:1, 2 * K : 3 * K], lhsT=ones_col_b[:, :1],
                        rhs=qf_cur, start=True, stop=True,
                        skip_group_check=True,
                    )
                    nc.tensor.matmul(
                        out=bcpair[:1, 3 * K : 4 * K], lhsT=ones_col_b[:, :1],
                        rhs=tmp_prev, start=True, stop=True,
                        skip_group_check=True,
                    )
                    nc.vector.tensor_copy(
                        out=csfb_sb[:1, ridx * 2 * K : (ridx + 1) * 2 * K],
                        in_=bcpair[:1, 2 * K : 4 * K],
                    )
                    rcs2 = srn.tile([1, 2 * K], dtype=bf16, tag="rcs")
                    with nc.allow_low_precision(reason="renorm scale; cancels"):
                        nc.vector.reciprocal(
                            out=rcs2[:], in_=bcpair[:1, 2 * K : 4 * K]
                        )
                    nc.tensor.matmul(
                        out=bcpair[:, 0:K], lhsT=ones_row_b[:1, :],
                        rhs=rcs2[:1, 0:K],
                        start=True, stop=True, skip_group_check=True,
                    )
                    nc.tensor.matmul(
                        out=bcpair[:, K : 2 * K], lhsT=ones_row_b[:1, :],
                        rhs=rcs2[:1, K : 2 * K],
                        start=True, stop=True, skip_group_check=True,
                    )
                    pend = bcpair[:, 0 : 2 * K]
            pkf = spfb.tile([K, 2 * K], dtype=f32, tag="pk")
            nc.tensor.matmul(
                out=pkf[:, K : 2 * K], lhsT=Wb_b[:], rhs=tmp_prev,
                start=True, stop=True, skip_group_check=True,
            )

            # ---------------- outputs ----------------
            qf_f = sq.tile([K, K], dtype=f32, tag="qff")
            nc.vector.tensor_copy(out=qf_f[:], in_=qf_cur)
            nc.sync.dma_start(out=qf_out_d[:, :], in_=qf_f[:])
            qb_f = sq.tile([K, K], dtype=f32, tag="qbf")
            nc.vector.tensor_copy(out=qb_f[:], in_=pkf[:, K : 2 * K])
            nc.sync.dma_start(out=qb_out_d[:, :], in_=qb_f[:])
            nc.sync.dma_start(out=csfb_out_d[:1, :], in_=csfb_sb[:])

    if not nc.is_finalized():
        nc.finalize()
    return nc


def _get_nc():
    if "nc" not in _CACHE:
        _CACHE["nc"] = _build_nc()
    return _CACHE["nc"]


def _pos(v):
    """DRAM row position of vocab row v in the block-permuted table."""
    g, rem = v // 1024, v % 1024
    b8, p = rem // K, rem % K
    return g * 1024 + p * 8 + b8


def _host_inputs(x, start_w, start_b, cluster_trans_w, emb_cluster_w,
                 cluster_vocab_w):
    x = np.asarray(x).astype(np.int64)
    tr = np.asarray(cluster_trans_w)[:, 0].reshape(K, K).astype(np.float32)
    m = tr.max(1, keepdims=True)
    e = np.exp(tr - m)
    texp = (e / e.sum(1, keepdims=True)).astype(np.float32)
    texpT = np.ascontiguousarray(texp.T)
    embT = np.ascontiguousarray(np.asarray(emb_cluster_w).astype(np.float32).T)
    import ml_dtypes
    voc = np.asarray(cluster_vocab_w).astype(np.float32)
    voc_p = np.ascontiguousarray(voc.T).astype(ml_dtypes.bfloat16)  # [K, V]
    p0 = np.exp(np.asarray(start_w)[:, 0].astype(np.float32)
                + np.asarray(start_b).astype(np.float32))

    in_maps = []
    for c in range(8):
        segs = [c, c + 8, c + 16, c + 24]
        toks = np.empty((K, L), np.int64)
        for si, j in enumerate(segs):
            # slot p = si*32 + n ; col r -> token x[n, j*L + r]
            toks[si * 32 : (si + 1) * 32, :] = x[:, j * L : (j + 1) * L]
        # first-use order over the gather emit order (0, L-1, 1, L-2, ...)
        emit = []
        for i in range(L // 2):
            emit += [i, L - 1 - i]
        seen = np.full(V, -1, np.int64)
        uorder = []
        for r in emit:
            for t in toks[:, r]:
                if seen[t] < 0:
                    seen[t] = len(uorder)
                    uorder.append(t)
        U = len(uorder)
        assert U <= TROWS
        # vocT column order: first-use tokens, then the rest, then pad
        rest = np.setdiff1d(np.arange(V), np.asarray(uorder), assume_unique=False)
        colord = np.concatenate([np.asarray(uorder, np.int64), rest])
        vocT_c = np.zeros((K, VP), voc_p.dtype)
        vocT_c[:, :V] = voc_p[:, colord]
        offs = _pos(seen[toks]).astype(np.int32)
        qf0 = np.ones((K, K), np.float32)
        if c == 0:
            qf0[:, 0:32] = p0[:, None]
        in_maps.append({
            "vocT": np.ascontiguousarray(vocT_c), "texp": texp, "texpT": texpT,
            "embT": embT, "qf0": qf0, "offs": offs,
        })
    return in_maps


def _assemble(res, x):
    """float64 telescoped assembly from per-core outputs"""
    x = np.asarray(x)

    def getF(j):
        c, si = j % 8, j // 8
        q = np.asarray(res[c]["qf_out"], np.float64)[:, si * 32 : (si + 1) * 32]
        cs = np.asarray(res[c]["csfb_out"], np.float64).reshape(NREN, 2, K)[:, 0]
        acc = np.log(cs[:, si * 32 : (si + 1) * 32]).sum(0)
        return q, acc

    def getB(j):
        c, si = j % 8, j // 8
        q = np.asarray(res[c]["qb_out"], np.float64)[:, si * 32 : (si + 1) * 32]
        cs = np.asarray(res[c]["csfb_out"], np.float64).reshape(NREN, 2, K)[:, 1]
        acc = np.log(cs[:, si * 32 : (si + 1) * 32]).sum(0)
        return q, acc

    f0, a0 = getF(0)
    b1, c1 = getB(1)
    lp = np.log((f0 * b1).sum(0)) + a0 + c1
    for j in range(1, S - 1):
        fj, aj = getF(j)
        bj1, cj1 = getB(j + 1)
        lp += np.log((fj * bj1).sum(0)) + cj1 - np.log(fj.sum(0))
    lp -= T * np.log(KAPPA)
    return np.float32(-lp.mean())


def kernel(x, start_w, start_b, cluster_trans_w, emb_cluster_w, cluster_vocab_w):
    from concourse.bass_utils import run_bass_kernel_spmd

    nc = _get_nc()
    in_maps = _host_inputs(x, start_w, start_b, cluster_trans_w,
                           emb_cluster_w, cluster_vocab_w)
    res = run_bass_kernel_spmd(nc, in_maps, list(range(8))).results
    return _assemble(res, x)
